# revision 72
# baseline (speedup 1.0000x reference)
"""Trainium2 Bass kernel for DGMG AddEdge log-prob (gnn_message_passing).

Math restructure (exact in real arithmetic):
    gate = sigmoid(hv @ Wg + bg)                    per node
    p    = hv @ (Wp @ We_g)                         per node (scalar!)
    logit_b = sum_{i in b} gate_i * p_i + hv[last_b] @ We_s + be
    out  = logsigmoid((2a - 1) * logit)
Only SCALAR segment sums are needed - the [B, G] segment_sum of the
reference is never materialized.  (bp = 0 in this problem, so the
gate-sum * (bp @ We_g) term vanishes; asserted host-side.)

Device pipeline per core (1024 graphs, <=63488 padded nodes, fp16):
  - hv stored feature-major [128 feat, NP nodes]; streamed once via 31
    DMA loads spread across the three DMA-capable engines (SP/ACT/Pool).
    In this machine's cost model a DMA occupies only its issuing engine,
    so the 49us of hv transfer runs at ~16.5us/engine.
  - PE: per 128-node tile, matmul(lhsT=hvT_tile, rhs=[-wg|w1]) ->
    psum [128 nodes, 2] = (-gate_logit, p).  Tiny output => tiny cost.
  - ACT: e = exp(-logit - bg) per 64-tile group; DVE: d = e+1,
    rc = 1/d, prod = p*rc  (gate = 1/(1+e); only ONE act table -
    exp/ln - is ever needed).
  - PE: per tile, matmul(lhsT=sel[128,4], rhs=prod[:,t]) -> psum[4,1]
    window partials into segP [4, 496].  sel is a host-baked one-hot
    over the <=4 graphs a 128-node tile can touch (seg_ids sorted),
    pre-multiplied by sgn = 2a-1 so the final sign comes for free.
  - Graphs are laid out g = 8p + c (partition p owns 8 consecutive
    graphs).  Partials of tiles < 384 drain to a tile-major DRAM
    scratch (row 4T+j) in two halves as each range completes; ONE
    indirect DMA then fetches, per partition, a W-element run starting
    at that partition's first tile (HW indirect-DMA semantics: one
    offset per partition, contiguous run).  A host-baked one-hot
    [128, 8, W] picks home+straddle partials per graph via a gpsimd
    multiply + DVE reduce.  All of this is off the critical path
    (drains+gather live on Pool, whose queue empties first).
  - Tail: partials of tiles >= 384 (owned only by graphs on partitions
    96..127) never touch DRAM: segP[:,384:] -> SBUF -> PE-transpose ->
    [112,4], then 32 tiny matmuls against host-baked 0/1 matrices
    combine them per graph in PSUM; one merged logsigmoid over all 128
    rows and a single store.  Emission is two-phase (all loads+dots
    first, gate/segment compute second) so no DMA queue is ever
    head-of-line blocked by data-gated compute; the tail group's loads
    are half-size so the last DMA completes early.
"""
import copy
import os
import sys

import numpy as np

for _p in ("/opt/trn_rl_repo",):
    if os.path.isdir(_p) and _p not in sys.path:
        sys.path.insert(0, _p)

import bass_rust
import concourse.bass as bass
import concourse.mybir as mybir
import concourse.tile as tile
from concourse.bass_utils import run_bass_kernel_spmd

F32 = mybir.dt.float32
F16 = mybir.dt.float16
F8 = mybir.dt.float8e4
I32 = mybir.dt.int32
AL = mybir.AluOpType
AF = mybir.ActivationFunctionType

NCORES = 8
N, B, D, G = 500_000, 8192, 128, 256
BL = B // NCORES           # graphs per core
TIL = 128                  # nodes per window tile
S = 4                      # segment window width per 128-node tile
NTIL = 496                 # tiles per core
NP = NTIL * TIL            # padded nodes per core (63488)
GT = 128                   # tiles per exp/divide group
# groups: 3 x 128 tiles, 1 x 64, then one 48-tile tail group
GDEF = [(0, 128), (128, 128), (256, 128), (384, 64), (448, 48)]
# per-group load widths in tiles: 16-tile (2048-node) loads for the body,
# 8-tile (1024-node) loads for the tail group so the final DMA's
# cost (and thus its data-ready time) is small
GLOADS = [[16] * 8, [16] * 8, [16] * 8, [16] * 4, [8] * 6]
NLOAD = sum(len(x) for x in GLOADS)
assert [sum(x) for x in GLOADS] == [n for _, n in GDEF]
NCH = BL // TIL            # 8 graphs per partition
TM0 = 384                  # tiles >= TM0 are combined on-chip (M path)
NTM = NTIL - TM0           # 112 on-chip tiles
PB0 = 96                   # partitions >= PB0 own graphs >= 768 (M path)
W = 24                     # gather run width (positions per partition)
VROWS = 2048

# hv-load engine pattern: 28 big loads (SP 10 / Pool 10 / ACT 8), then the
# six half-size tail loads (SP 3 / ACT 2 / Pool 1) interleaved so each of
# the last three groups' pairs lands on two different engines
LOAD_ENGS = (["sync", "gpsimd", "scalar"] * 8 + ["sync", "gpsimd"] * 2 +
             ["sync", "scalar", "sync", "scalar", "scalar", "sync"])
assert len(LOAD_ENGS) == NLOAD

LAST_RESULTS = None

_WS_CTR = [0]


def split_sync_waits(nc, maxw=1):
    """This walrus build rejects instructions with more than one semaphore
    wait; hoist excess waits onto injected same-engine NoOps."""
    for fn in nc.m.functions:
        for bb in fn.blocks:
            out, changed = [], False
            for inst in bb.instructions:
                si = inst.sync_info
                if si is not None and si.on_wait and len(si.on_wait) > maxw:
                    SI = type(si)
                    waits = list(si.on_wait)
                    extra, keep = waits[:-maxw], waits[-maxw:]
                    for k in range(0, len(extra), maxw):
                        nop = mybir.InstNoOp(
                            name=f"waitsplit_{_WS_CTR[0]}", ins=[], outs=[])
                        _WS_CTR[0] += 1
                        nop.engine = inst.engine
                        nop.bass_nofuse = True
                        nop.sync_info = SI(
                            on_wait=extra[k:k + maxw], on_update=[])
                        out.append(nop)
                    inst.sync_info = SI(
                        on_wait=keep, on_update=list(si.on_update or []))
                    changed = True
                out.append(inst)
            if changed:
                bb.instructions = out
    return nc


def _dram_view(handle, offset_elems, dims):
    """AP over a DRAM tensor with explicit [step, count] dims (element units
    over the row-major flattened tensor)."""
    ap = copy.copy(handle[:, :] if len(handle.shape) > 1 else handle[:])
    ap.offset = offset_elems
    ap.ap = bass_rust.VecI64Pair(dims)
    return ap


def _bcast_mid(ap, n):
    """[P, W] AP -> [P, n, W] with a 0-stride middle dim (broadcast)."""
    a = copy.copy(ap)
    dims = [list(x) for x in ap.ap]
    assert len(dims) == 2
    a.ap = bass_rust.VecI64Pair([dims[0], [0, n], dims[1]])
    return a


def _logsigmoid_chain(nc, pool, x_ap, np_, nf, tag):
    """min(x,0) - log1p(exp(-|x|)) on a [np_, nf] slice; returns out tile."""
    mn = pool.tile([np_, nf], F32, name=f"mn{tag}")
    nc.vector.tensor_scalar_min(mn[:], x_ap, 0.0)
    mx = pool.tile([np_, nf], F32, name=f"mx{tag}")
    nc.vector.tensor_scalar_max(mx[:], x_ap, 0.0)
    nax = pool.tile([np_, nf], F32, name=f"nax{tag}")
    nc.vector.tensor_sub(nax[:], mn[:], mx[:])
    ee = pool.tile([np_, nf], F32, name=f"ee{tag}")
    nc.scalar.activation(ee[:], nax[:], AF.Exp)
    lp = pool.tile([np_, nf], F32, name=f"lp{tag}")
    nc.scalar.activation(lp[:], ee[:], AF.Ln, bias=1.0)
    ob = pool.tile([np_, nf], F32, name=f"ob{tag}")
    nc.vector.tensor_sub(ob[:], mn[:], lp[:])
    return ob


def _build(bg0: float, be0: float, c1: float, debug: bool = False) -> bass.Bass:
    nc = bass.Bass()
    if debug:
        vdbg_d = nc.declare_dram_parameter("vdbg", [VROWS, 1], F32, isOutput=True)
        vvdbg_d = nc.declare_dram_parameter("vvdbg", [TIL, W], F32, isOutput=True)
        sbtdbg_d = nc.declare_dram_parameter("sbtdbg", [NTM, S], F32, isOutput=True)
    hv_d = nc.declare_dram_parameter("hvT", [TIL, NP], F16, isOutput=False)
    sel_d = nc.declare_dram_parameter("sel", [TIL, NTIL * S], F8, isOutput=False)
    src_d = nc.declare_dram_parameter("srcT", [TIL, BL], F16, isOutput=False)
    mt_d = nc.declare_dram_parameter("mt", [NTM, NCH * S * (TIL - PB0)], F8,
                                     isOutput=False)
    oh_d = nc.declare_dram_parameter("oh", [TIL, NCH * W], F16, isOutput=False)
    # packed fp16 consts: cols 0:2 = [-wg | w1], 2:3 = wes, 3:7 = eye4
    pk_d = nc.declare_dram_parameter("pk", [TIL, 7], F16, isOutput=False)
    idx_d = nc.declare_dram_parameter("idx", [TIL, 1], I32, isOutput=False)
    out_d = nc.declare_dram_parameter("out", [BL, 1], F32, isOutput=True)
    virt_d = nc.dram_tensor("virt", [VROWS, 1], F32)

    with tile.TileContext(nc) as tc:
        with (
            tc.tile_pool(name="consts", bufs=1) as cpool,
            tc.tile_pool(name="hvp", bufs=12) as hvpool,
            tc.tile_pool(name="small", bufs=3) as spool,
            tc.tile_pool(name="stg", bufs=1) as gpool,
            tc.tile_pool(name="tailp", bufs=1) as tpool,
            tc.tile_pool(name="pdots", bufs=4, space="PSUM") as pdots,
            tc.tile_pool(name="pseg", bufs=1, space="PSUM") as pseg,
            tc.tile_pool(name="psrc", bufs=1, space="PSUM") as psrc,
            tc.tile_pool(name="ptail", bufs=1, space="PSUM") as ptail,
        ):
            # ---- consts ----
            pk_t = cpool.tile([TIL, 7], F16)
            nc.scalar.dma_start(pk_t[:], pk_d[:])
            # zero-fill the DRAM scratch (gather runs may cross into
            # undrained rows whose one-hot weight is 0 - keep them finite)
            zf = cpool.tile([TIL, VROWS // TIL], F32)
            nc.gpsimd.memset(zf[:], 0.0)
            nvc = VROWS // TIL
            zdr = nc.gpsimd.dma_start(
                _dram_view(virt_d, 0, [[nvc, TIL], [1, nvc]]), zf[:])
            sel_t = cpool.tile([TIL, NTIL * S], F8)
            nc.scalar.dma_start(sel_t[:], sel_d[:])
            src_t = cpool.tile([TIL, BL], F16)
            mt_t = cpool.tile([NTM, NCH * S * (TIL - PB0)], F8)
            oh_t = cpool.tile([TIL, NCH * W], F16)
            idx_t = cpool.tile([TIL, 1], I32)

            segP = pseg.tile([S, NTIL], F32, name="segP")
            # one bank shared: srcP cols 0:8, on-chip combine cols 8:16,
            # and the three small groups' dots at cols 16:112
            shP = psrc.tile([TIL, 112], F32, name="shP")

            seg_mms = []
            drains = [zdr]
            load_i = 0
            NQB = TIL - PB0
            outPB = shP[0:NQB, NCH:2 * NCH]

            # ---- emission phase A: all hv loads + PE dots ----
            # (keeps each DMA engine's queue free of compute-gated work so
            # the loads run back-to-back; one psum dots tile per group)
            Pgs = []
            for g, (t0, ntile) in enumerate(GDEF):
                if ntile > 48:
                    Pg = pdots.tile([TIL, 2 * GT], F32, name="Pg")[:]
                else:
                    Pg = shP[:, 16:16 + 2 * ntile]
                Pgs.append(Pg)
                hv_tiles = []
                ltil = 0
                for lw in GLOADS[g]:
                    hv_t = hvpool.tile([TIL, 16 * TIL], F16, name="hv")
                    eng = getattr(nc, LOAD_ENGS[load_i])
                    off = (t0 + ltil) * TIL
                    eng.dma_start(hv_t[:, :lw * TIL],
                                  hv_d[:, off:off + lw * TIL])
                    hv_tiles.append((hv_t, ltil, lw))
                    ltil += lw
                    load_i += 1
                if g == 1:
                    nc.gpsimd.dma_start(src_t[:], src_d[:])
                if g == 2:
                    nc.gpsimd.dma_start(mt_t[:], mt_d[:])
                    nc.gpsimd.dma_start(oh_t[:], oh_d[:])
                    nc.gpsimd.dma_start(idx_t[:], idx_d[:])

                # dots: psum col layout interleaved (g, p) per tile
                for hv_t, ltil, lw in hv_tiles:
                    for u in range(lw):
                        t = ltil + u
                        nc.tensor.matmul(
                            Pg[:, 2 * t:2 * t + 2],
                            lhsT=hv_t[:, TIL * u:TIL * (u + 1)],
                            rhs=pk_t[:, 0:2], start=True, stop=True)
                if g == 2:
                    # src term: sgn * (src @ wes); column 128c+p holds the
                    # graph 8p+c so srcP[p, c] lands in graph layout
                    for c in range(NCH):
                        nc.tensor.matmul(
                            shP[:, c:c + 1],
                            lhsT=src_t[:, TIL * c:TIL * (c + 1)],
                            rhs=pk_t[:, 2:3], start=True, stop=True)

            # ---- emission phase B: gate/products/segment partials ----
            for g, (t0, ntile) in enumerate(GDEF):
                Pg3 = Pgs[g].rearrange("p (c two) -> p c two", two=2)
                e_t = spool.tile([TIL, GT], F32, name="e")
                nc.scalar.activation(e_t[:, :ntile], Pg3[:, :ntile, 0],
                                     AF.Exp, bias=-bg0)
                d_t = spool.tile([TIL, GT], F32, name="d")
                nc.vector.tensor_scalar_add(d_t[:, :ntile], e_t[:, :ntile], 1.0)
                rc_t = spool.tile([TIL, GT], F32, name="rc")
                nc.vector.reciprocal(rc_t[:, :ntile], d_t[:, :ntile])
                prod = spool.tile([TIL, GT], F16, name="prod")
                nc.vector.tensor_tensor(
                    out=prod[:, :ntile], in0=Pg3[:, :ntile, 1],
                    in1=rc_t[:, :ntile], op=AL.mult)

                for t in range(ntile):
                    T = t0 + t
                    mm = nc.tensor.matmul(
                        segP[:, T:T + 1], lhsT=sel_t[:, S * T:S * T + S],
                        rhs=prod[:, t:t + 1], start=True, stop=True)
                    seg_mms.append(mm)

                # drain early halves (tiles < TM0), tile-major rows 4T+j
                for k, (qlo, qhi) in enumerate(((0, 256), (256, TM0))):
                    if t0 + ntile == qhi:
                        stg = gpool.tile([S, 256], F32, name=f"stg{k}")
                        cp = nc.vector.tensor_copy(
                            stg[:, :qhi - qlo], segP[:, qlo:qhi])
                        for mm in seg_mms:
                            tile.add_dep_helper(cp.ins, mm.ins)
                        dr = nc.gpsimd.dma_start(
                            _dram_view(virt_d, S * qlo,
                                       [[1, S], [S, qhi - qlo]]),
                            stg[:, :qhi - qlo])
                        drains.append(dr)

                if t0 + ntile == TM0:
                    # one run-gather: partition p gets virt[4*t0(p) .. +W)
                    vv = tpool.tile([TIL, W], F32, name="vv")
                    gth = nc.gpsimd.indirect_dma_start(
                        out=vv[:], out_offset=None, in_=virt_d[:],
                        in_offset=bass.IndirectOffsetOnAxis(
                            ap=idx_t[:], axis=0))
                    for dr in drains:
                        tile.add_dep_helper(gth.ins, dr.ins)


            # ---- on-chip combine for tiles >= TM0 (graph rows PB0..127)
            stg4 = gpool.tile([S, NTM], F16, name="stg4")
            cp4 = nc.scalar.activation(stg4[:], segP[:, TM0:NTIL], AF.Copy)
            for mm in seg_mms:
                tile.add_dep_helper(cp4.ins, mm.ins)
            ptr = ptail.tile([NTM, S], F16, name="ptr")
            nc.tensor.transpose(ptr[:], stg4[:], pk_t[0:S, 3:7])
            sbT = gpool.tile([NTM, S], F16, name="sbT")
            nc.scalar.activation(sbT[:], ptr[:], AF.Copy)
            for c in range(NCH):
                for j in range(S):
                    blk = NQB * (S * c + j)
                    nc.tensor.matmul(
                        shP[0:NQB, NCH + c:NCH + c + 1],
                        lhsT=mt_t[:, blk:blk + NQB],
                        rhs=sbT[:, j:j + 1],
                        start=(j == 0), stop=(j == S - 1))

            # ---- select partials per graph: s[p,c] = sum_k vv[p,k]*oh[p,c,k]
            # (mult on gpsimd: Pool is idle once its loads finish; fp16
            # intermediates unlock the DVE 2x reduce mode)
            tsel = tpool.tile([TIL, NCH * W], F16, name="tsel")
            nc.gpsimd.tensor_tensor(
                out=tsel[:].rearrange("p (c k) -> p c k", c=NCH),
                in0=_bcast_mid(vv[:], NCH),
                in1=oh_t[:].rearrange("p (c k) -> p c k", c=NCH),
                op=AL.mult)
            s_t = tpool.tile([TIL, NCH], F32, name="s")
            nc.vector.tensor_reduce(
                out=s_t[:],
                in_=tsel[:].rearrange("p (c k) -> p c k", c=NCH),
                axis=mybir.AxisListType.X, op=AL.add)

            # x for all 1024 graphs, then accumulate the on-chip combine
            # into rows >= PB0 in place; single merged logsigmoid + store
            xF = tpool.tile([TIL, NCH], F32, name="xF")
            nc.vector.tensor_add(xF[:], s_t[:], shP[:, 0:NCH])
            if be0 != 0.0:
                xb2 = tpool.tile([TIL, NCH], F32, name="xb2")
                nc.vector.tensor_scalar_add(xb2[:], xF[:], be0)
                xF = xb2
            nc.vector.tensor_add(xF[PB0:TIL, :], xF[PB0:TIL, :], outPB)
            ob = _logsigmoid_chain(nc, tpool, xF[:], TIL, NCH, "F")
            outF = _dram_view(out_d, 0, [[NCH, TIL], [1, NCH]])
            nc.sync.dma_start(outF, ob[:])

            if debug:
                vcp = tpool.tile([TIL, W], F32, name="vcp")
                nc.vector.tensor_copy(vcp[:], vv[:])
                nc.sync.dma_start(vvdbg_d[:, :], vcp[:])
                vload = tpool.tile([TIL, nvc], F32, name="vload")
                gd = nc.gpsimd.dma_start(
                    vload[:], _dram_view(virt_d, 0, [[nvc, TIL], [1, nvc]]))
                for dr in drains:
                    tile.add_dep_helper(gd.ins, dr.ins)
                nc.sync.dma_start(
                    _dram_view(vdbg_d, 0, [[nvc, TIL], [1, nvc]]), vload[:])
                sbc = tpool.tile([NTM, S], F32, name="sbc")
                nc.vector.tensor_copy(sbc[:], sbT[:])
                nc.sync.dma_start(sbtdbg_d[:, :], sbc[:])
    return nc


def _prep_core(hv, seg_ids, last_idx, a, m):
    lo = int(np.searchsorted(seg_ids, m * BL, "left"))
    hi = int(np.searchsorted(seg_ids, (m + 1) * BL, "left"))
    nloc = hi - lo
    assert nloc <= NP - TIL, f"core {m}: {nloc} nodes > capacity"
    seg_loc = seg_ids[lo:hi].astype(np.int64) - m * BL
    sgn = (2 * a[m * BL:(m + 1) * BL] - 1).astype(np.float32)

    hvT = np.zeros((TIL, NP), np.float16)
    hvT[:, :nloc] = hv[lo:hi].astype(np.float16).T

    nrt = (nloc + TIL - 1) // TIL
    b = np.zeros(NTIL, np.int64)
    b[:nrt] = seg_loc[np.arange(nrt) * TIL]
    rel = seg_loc - b[np.arange(nloc) // TIL]
    assert rel.min() >= 0 and rel.max() < S, f"window overflow: {rel.max()}"

    import ml_dtypes
    sel = np.zeros((TIL, NTIL * S), ml_dtypes.float8_e4m3)
    ii = np.arange(nloc)
    sel[ii % TIL, S * (ii // TIL) + rel] = sgn[seg_loc]

    rr = np.arange(BL, dtype=np.int64)
    firsts = np.searchsorted(seg_loc, rr, "left")
    lasts = np.searchsorted(seg_loc, rr + 1, "left")
    nonempty = firsts < lasts
    th = firsts // TIL
    tl = np.maximum(lasts - 1, 0) // TIL
    assert np.all((tl - th)[nonempty] <= 1), "segment spans >2 tiles"
    j1 = rr - b[th]
    assert np.all((j1[nonempty] >= 0) & (j1[nonempty] < S))
    straddle = nonempty & (tl > th)
    assert np.all(b[tl[straddle]] == rr[straddle])
    # graphs on partitions < PB0 (g < 8*PB0) live entirely in tiles < TM0
    assert np.all(tl[nonempty & (rr < NCH * PB0)] < TM0), \
        f"core {m}: early graph owns a late tile"

    # run offsets: partition p covers graphs 8p..8p+7 from tile t0(p)
    t0p = th[NCH * np.arange(TIL)]
    idx = (S * t0p).astype(np.int32).reshape(TIL, 1)
    # one-hot selector oh[p, c, k]: k = 4*(t - t0p) + j for each early
    # partial (home j1 at th; straddle slot 0 at tl)
    oh = np.zeros((TIL, NCH, W), np.float16)
    pp, cc = rr // NCH, rr % NCH
    he = nonempty & (th < TM0)
    k1 = S * (th - t0p[pp]) + j1
    assert np.all((k1[he] >= 0) & (k1[he] < W)), "run width overflow"
    oh[pp[he], cc[he], k1[he]] += 1.0
    se = straddle & (tl < TM0)
    k2 = S * (tl - t0p[pp])
    assert np.all((k2[se] >= 0) & (k2[se] < W)), "run width overflow"
    oh[pp[se], cc[se], k2[se]] += 1.0
    oh = oh.reshape(TIL, NCH * W)

    # M combine for on-chip partials (tiles >= TM0, graphs g >= 8*PB0):
    # block col NQB*(4c + j) + (p - PB0)
    NQB = TIL - PB0
    mt = np.zeros((NTM, NCH * S * NQB), ml_dtypes.float8_e4m3)
    lh = nonempty & (th >= TM0)
    assert np.all(rr[lh] >= NCH * PB0)
    mt[th[lh] - TM0, NQB * (S * cc[lh] + j1[lh]) + pp[lh] - PB0] = 1.0
    ls = straddle & (tl >= TM0)
    assert np.all(rr[ls] >= NCH * PB0)
    mt[tl[ls] - TM0, NQB * (S * cc[ls] + 0) + pp[ls] - PB0] = 1.0

    # srcT column 128c+p holds (sign-folded) src row of graph 8p+c
    src = hv[last_idx[m * BL:(m + 1) * BL]].astype(np.float32) * sgn[:, None]
    srcT = np.zeros((TIL, BL), np.float16)
    gg = np.arange(BL)
    srcT[:, TIL * (gg % NCH) + gg // NCH] = src.T.astype(np.float16)
    return hvT, sel, srcT, mt, oh, idx


def prep_all(hv, Wg, bg, Wp, bp, We, be, seg_ids, last_idx, a):
    hv = np.asarray(hv, dtype=np.float32)
    Wg = np.asarray(Wg, dtype=np.float32)
    bg = np.asarray(bg, dtype=np.float32)
    Wp = np.asarray(Wp, dtype=np.float32)
    bp = np.asarray(bp, dtype=np.float32)
    We = np.asarray(We, dtype=np.float32)
    be = np.asarray(be, dtype=np.float32)
    seg_ids = np.asarray(seg_ids)
    last_idx = np.asarray(last_idx)
    a = np.asarray(a)

    w1 = (Wp @ We[:G]).astype(np.float32)[:, 0]        # [128]
    wes = We[G:, 0].astype(np.float32)                 # [128]
    c1 = float(bp @ We[:G, 0])
    bg0, be0 = float(bg[0]), float(be[0])
    # bp is zeros in this problem's setup_inputs
    assert c1 == 0.0, "c1 != 0 path not implemented"

    pk = np.zeros((TIL, 7), np.float16)
    pk[:, 0] = -Wg[:, 0]
    pk[:, 1] = w1
    pk[:, 2] = wes
    pk[:S, 3:7] = np.eye(S, dtype=np.float16)

    in_maps = []
    for m in range(NCORES):
        hvT, sel, srcT, mt, oh, idx = _prep_core(hv, seg_ids, last_idx, a, m)
        in_maps.append({
            "hvT": hvT, "sel": sel, "srcT": srcT, "mt": mt,
            "oh": oh, "idx": idx, "pk": pk,
        })
    return in_maps, bg0, be0, c1


def _unpermute(out_flat):
    """Device graph order is g = 8p + c stored at flat index 8p+c == g."""
    return out_flat


def kernel(hv, Wg, bg, Wp, bp, We, be, seg_ids, last_idx, a):
    global LAST_RESULTS
    in_maps, bg0, be0, c1 = prep_all(
        hv, Wg, bg, Wp, bp, We, be, seg_ids, last_idx, a)
    nc = _build(bg0, be0, c1)
    split_sync_waits(nc, maxw=1)
    res = run_bass_kernel_spmd(nc, in_maps, core_ids=list(range(NCORES)))
    LAST_RESULTS = res
    out = np.concatenate([np.asarray(res.results[i]["out"]) for i in range(NCORES)], axis=0)
    return out.astype(np.float32)


# revision 77
# speedup vs baseline: 1.0014x; 1.0014x over previous
"""Trainium2 Bass kernel for DGMG AddEdge log-prob (gnn_message_passing).

Math restructure (exact in real arithmetic):
    gate = sigmoid(hv @ Wg + bg)                    per node
    p    = hv @ (Wp @ We_g)                         per node (scalar!)
    logit_b = sum_{i in b} gate_i * p_i + hv[last_b] @ We_s + be
    out  = logsigmoid((2a - 1) * logit)
Only SCALAR segment sums are needed - the [B, G] segment_sum of the
reference is never materialized.  (bp = 0 in this problem, so the
gate-sum * (bp @ We_g) term vanishes; asserted host-side.)

Device pipeline per core (1024 graphs, <=63488 padded nodes, fp16):
  - hv stored feature-major [128 feat, NP nodes]; streamed once via 31
    DMA loads spread across the three DMA-capable engines (SP/ACT/Pool).
    In this machine's cost model a DMA occupies only its issuing engine,
    so the 49us of hv transfer runs at ~16.5us/engine.
  - PE: per 128-node tile, matmul(lhsT=hvT_tile, rhs=[-wg|w1]) ->
    psum [128 nodes, 2] = (-gate_logit, p).  Tiny output => tiny cost.
  - ACT: e = exp(-logit - bg) per 64-tile group; DVE: d = e+1,
    rc = 1/d, prod = p*rc  (gate = 1/(1+e); only ONE act table -
    exp/ln - is ever needed).
  - PE: per tile, matmul(lhsT=sel[128,4], rhs=prod[:,t]) -> psum[4,1]
    window partials into segP [4, 496].  sel is a host-baked one-hot
    over the <=4 graphs a 128-node tile can touch (seg_ids sorted),
    pre-multiplied by sgn = 2a-1 so the final sign comes for free.
  - Graphs are laid out g = 8p + c (partition p owns 8 consecutive
    graphs).  Partials of tiles < 384 drain to a tile-major DRAM
    scratch (row 4T+j) in two halves as each range completes; ONE
    indirect DMA then fetches, per partition, a W-element run starting
    at that partition's first tile (HW indirect-DMA semantics: one
    offset per partition, contiguous run).  A host-baked one-hot
    [128, 8, W] picks home+straddle partials per graph via a gpsimd
    multiply + DVE reduce.  All of this is off the critical path
    (drains+gather live on Pool, whose queue empties first).
  - Tail: partials of tiles >= 384 (owned only by graphs on partitions
    96..127) never touch DRAM: segP[:,384:] -> SBUF -> PE-transpose ->
    [112,4], then 32 tiny matmuls against host-baked 0/1 matrices
    combine them per graph in PSUM; one merged logsigmoid over all 128
    rows and a single store.  Emission is two-phase (all loads+dots
    first, gate/segment compute second) so no DMA queue is ever
    head-of-line blocked by data-gated compute; the tail group's loads
    are half-size so the last DMA completes early.
"""
import copy
import os
import sys

import numpy as np

for _p in ("/opt/trn_rl_repo",):
    if os.path.isdir(_p) and _p not in sys.path:
        sys.path.insert(0, _p)

import bass_rust
import concourse.bass as bass
import concourse.mybir as mybir
import concourse.tile as tile
from concourse.bass_utils import run_bass_kernel_spmd

F32 = mybir.dt.float32
F16 = mybir.dt.float16
F8 = mybir.dt.float8e4
I32 = mybir.dt.int32
AL = mybir.AluOpType
AF = mybir.ActivationFunctionType

NCORES = 8
N, B, D, G = 500_000, 8192, 128, 256
BL = B // NCORES           # graphs per core
TIL = 128                  # nodes per window tile
S = 4                      # segment window width per 128-node tile
NTIL = 496                 # tiles per core
NP = NTIL * TIL            # padded nodes per core (63488)
GT = 128                   # tiles per exp/divide group
# groups: 3 x 128 tiles, 1 x 64, then one 48-tile tail group
GDEF = [(0, 128), (128, 128), (256, 128), (384, 64), (448, 48)]
# per-group load widths in tiles: 16-tile (2048-node) loads for the body,
# 8-tile (1024-node) loads for the tail group so the final DMA's
# cost (and thus its data-ready time) is small
GLOADS = [[16] * 8, [16] * 8, [16] * 8, [16] * 4, [8] * 6]
NLOAD = sum(len(x) for x in GLOADS)
assert [sum(x) for x in GLOADS] == [n for _, n in GDEF]
NCH = BL // TIL            # 8 graphs per partition
TM0 = 384                  # tiles >= TM0 are combined on-chip (M path)
NTM = NTIL - TM0           # 112 on-chip tiles
PB0 = 96                   # partitions >= PB0 own graphs >= 768 (M path)
W = 24                     # gather run width (positions per partition)
VROWS = 2048

# hv-load engine pattern: 28 big loads (SP 10 / Pool 10 / ACT 8), then the
# six half-size tail loads (SP 3 / ACT 2 / Pool 1) interleaved so each of
# the last three groups' pairs lands on two different engines
LOAD_ENGS = (["sync", "gpsimd", "scalar"] * 8 + ["sync", "gpsimd"] * 2 +
             ["sync", "scalar", "sync", "scalar", "scalar", "sync"])
assert len(LOAD_ENGS) == NLOAD

LAST_RESULTS = None

_WS_CTR = [0]


def split_sync_waits(nc, maxw=1):
    """This walrus build rejects instructions with more than one semaphore
    wait; hoist excess waits onto injected same-engine NoOps."""
    for fn in nc.m.functions:
        for bb in fn.blocks:
            out, changed = [], False
            for inst in bb.instructions:
                si = inst.sync_info
                if si is not None and si.on_wait and len(si.on_wait) > maxw:
                    SI = type(si)
                    waits = list(si.on_wait)
                    extra, keep = waits[:-maxw], waits[-maxw:]
                    for k in range(0, len(extra), maxw):
                        nop = mybir.InstNoOp(
                            name=f"waitsplit_{_WS_CTR[0]}", ins=[], outs=[])
                        _WS_CTR[0] += 1
                        nop.engine = inst.engine
                        nop.bass_nofuse = True
                        nop.sync_info = SI(
                            on_wait=extra[k:k + maxw], on_update=[])
                        out.append(nop)
                    inst.sync_info = SI(
                        on_wait=keep, on_update=list(si.on_update or []))
                    changed = True
                out.append(inst)
            if changed:
                bb.instructions = out
    return nc


def _dram_view(handle, offset_elems, dims):
    """AP over a DRAM tensor with explicit [step, count] dims (element units
    over the row-major flattened tensor)."""
    ap = copy.copy(handle[:, :] if len(handle.shape) > 1 else handle[:])
    ap.offset = offset_elems
    ap.ap = bass_rust.VecI64Pair(dims)
    return ap


def _bcast_mid(ap, n):
    """[P, W] AP -> [P, n, W] with a 0-stride middle dim (broadcast)."""
    a = copy.copy(ap)
    dims = [list(x) for x in ap.ap]
    assert len(dims) == 2
    a.ap = bass_rust.VecI64Pair([dims[0], [0, n], dims[1]])
    return a


def _logsigmoid_chain(nc, pool, x_ap, np_, nf, tag):
    """min(x,0) - log1p(exp(-|x|)) on a [np_, nf] slice; returns out tile."""
    mn = pool.tile([np_, nf], F32, name=f"mn{tag}")
    nc.vector.tensor_scalar_min(mn[:], x_ap, 0.0)
    mx = pool.tile([np_, nf], F32, name=f"mx{tag}")
    nc.vector.tensor_scalar_max(mx[:], x_ap, 0.0)
    nax = pool.tile([np_, nf], F32, name=f"nax{tag}")
    nc.vector.tensor_sub(nax[:], mn[:], mx[:])
    ee = pool.tile([np_, nf], F32, name=f"ee{tag}")
    nc.scalar.activation(ee[:], nax[:], AF.Exp)
    lp = pool.tile([np_, nf], F32, name=f"lp{tag}")
    nc.scalar.activation(lp[:], ee[:], AF.Ln, bias=1.0)
    ob = pool.tile([np_, nf], F32, name=f"ob{tag}")
    nc.vector.tensor_sub(ob[:], mn[:], lp[:])
    return ob


def _build(bg0: float, be0: float, c1: float, debug: bool = False) -> bass.Bass:
    nc = bass.Bass()
    if debug:
        vdbg_d = nc.declare_dram_parameter("vdbg", [VROWS, 1], F32, isOutput=True)
        vvdbg_d = nc.declare_dram_parameter("vvdbg", [TIL, W], F32, isOutput=True)
        sbtdbg_d = nc.declare_dram_parameter("sbtdbg", [NTM, S], F32, isOutput=True)
    hv_d = nc.declare_dram_parameter("hvT", [TIL, NP], F16, isOutput=False)
    sel_d = nc.declare_dram_parameter("sel", [TIL, NTIL * S], F8, isOutput=False)
    src_d = nc.declare_dram_parameter("srcT", [TIL, BL], F16, isOutput=False)
    mt_d = nc.declare_dram_parameter("mt", [NTM, NCH * S * (TIL - PB0)], F8,
                                     isOutput=False)
    oh_d = nc.declare_dram_parameter("oh", [TIL, NCH * W], F16, isOutput=False)
    # packed fp16 consts: cols 0:2 = [-wg | w1], 2:3 = wes, 3:7 = eye4
    pk_d = nc.declare_dram_parameter("pk", [TIL, 7], F16, isOutput=False)
    idx_d = nc.declare_dram_parameter("idx", [TIL, 1], I32, isOutput=False)
    out_d = nc.declare_dram_parameter("out", [BL, 1], F32, isOutput=True)
    virt_d = nc.dram_tensor("virt", [VROWS, 1], F32)

    with tile.TileContext(nc) as tc:
        with (
            tc.tile_pool(name="consts", bufs=1) as cpool,
            tc.tile_pool(name="hvp", bufs=12) as hvpool,
            tc.tile_pool(name="small", bufs=3) as spool,
            tc.tile_pool(name="stg", bufs=1) as gpool,
            tc.tile_pool(name="tailp", bufs=1) as tpool,
            tc.tile_pool(name="pdots", bufs=4, space="PSUM") as pdots,
            tc.tile_pool(name="pseg", bufs=1, space="PSUM") as pseg,
            tc.tile_pool(name="psrc", bufs=1, space="PSUM") as psrc,
            tc.tile_pool(name="ptail", bufs=1, space="PSUM") as ptail,
        ):
            # ---- consts ----
            pk_t = cpool.tile([TIL, 7], F16)
            nc.scalar.dma_start(pk_t[:], pk_d[:])
            # zero-fill the DRAM scratch (gather runs may cross into
            # undrained rows whose one-hot weight is 0 - keep them finite)
            zf = cpool.tile([TIL, VROWS // TIL], F32)
            nc.gpsimd.memset(zf[:], 0.0)
            nvc = VROWS // TIL
            zdr = nc.gpsimd.dma_start(
                _dram_view(virt_d, 0, [[nvc, TIL], [1, nvc]]), zf[:])
            sel_t = cpool.tile([TIL, NTIL * S], F8)
            nc.scalar.dma_start(sel_t[:], sel_d[:])
            src_t = cpool.tile([TIL, BL], F16)
            mt_t = cpool.tile([NTM, NCH * S * (TIL - PB0)], F8)
            oh_t = cpool.tile([TIL, NCH * W], F16)
            idx_t = cpool.tile([TIL, 1], I32)

            segP = pseg.tile([S, NTIL], F32, name="segP")
            # one bank shared: srcP cols 0:8, on-chip combine cols 8:16,
            # and the three small groups' dots at cols 16:112
            shP = psrc.tile([TIL, 112], F32, name="shP")

            seg_mms = []
            drains = [zdr]
            load_i = 0
            NQB = TIL - PB0
            outPB = shP[0:NQB, NCH:2 * NCH]

            # ---- emission phase A: all hv loads + PE dots ----
            # (keeps each DMA engine's queue free of compute-gated work so
            # the loads run back-to-back; one psum dots tile per group)
            Pgs = []
            for g, (t0, ntile) in enumerate(GDEF):
                if ntile > 48:
                    Pg = pdots.tile([TIL, 2 * GT], F32, name="Pg")[:]
                else:
                    Pg = shP[:, 16:16 + 2 * ntile]
                Pgs.append(Pg)
                hv_tiles = []
                ltil = 0
                for lw in GLOADS[g]:
                    hv_t = hvpool.tile([TIL, 16 * TIL], F16, name="hv")
                    eng = getattr(nc, LOAD_ENGS[load_i])
                    off = (t0 + ltil) * TIL
                    eng.dma_start(hv_t[:, :lw * TIL],
                                  hv_d[:, off:off + lw * TIL])
                    hv_tiles.append((hv_t, ltil, lw))
                    ltil += lw
                    load_i += 1
                if g == 1:
                    nc.gpsimd.dma_start(src_t[:], src_d[:])
                if g == 2:
                    nc.gpsimd.dma_start(mt_t[:], mt_d[:])
                    nc.gpsimd.dma_start(oh_t[:], oh_d[:])
                    nc.gpsimd.dma_start(idx_t[:], idx_d[:])

                # dots: psum col layout interleaved (g, p) per tile
                for hv_t, ltil, lw in hv_tiles:
                    for u in range(lw):
                        t = ltil + u
                        nc.tensor.matmul(
                            Pg[:, 2 * t:2 * t + 2],
                            lhsT=hv_t[:, TIL * u:TIL * (u + 1)],
                            rhs=pk_t[:, 0:2], start=True, stop=True)
                if g == 2:
                    # src term: sgn * (src @ wes); column 128c+p holds the
                    # graph 8p+c so srcP[p, c] lands in graph layout
                    for c in range(NCH):
                        nc.tensor.matmul(
                            shP[:, c:c + 1],
                            lhsT=src_t[:, TIL * c:TIL * (c + 1)],
                            rhs=pk_t[:, 2:3], start=True, stop=True)

            # ---- emission phase B: gate/products/segment partials ----
            for g, (t0, ntile) in enumerate(GDEF):
                Pg3 = Pgs[g].rearrange("p (c two) -> p c two", two=2)
                e_t = spool.tile([TIL, GT], F32, name="e")
                nc.scalar.activation(e_t[:, :ntile], Pg3[:, :ntile, 0],
                                     AF.Exp, bias=-bg0)
                d_t = spool.tile([TIL, GT], F32, name="d")
                nc.vector.tensor_scalar_add(d_t[:, :ntile], e_t[:, :ntile], 1.0)
                rc_t = spool.tile([TIL, GT], F32, name="rc")
                nc.vector.reciprocal(rc_t[:, :ntile], d_t[:, :ntile])
                prod = spool.tile([TIL, GT], F16, name="prod")
                nc.vector.tensor_tensor(
                    out=prod[:, :ntile], in0=Pg3[:, :ntile, 1],
                    in1=rc_t[:, :ntile], op=AL.mult)

                for t in range(ntile):
                    T = t0 + t
                    mm = nc.tensor.matmul(
                        segP[:, T:T + 1], lhsT=sel_t[:, S * T:S * T + S],
                        rhs=prod[:, t:t + 1], start=True, stop=True)
                    seg_mms.append(mm)

                # drain early halves (tiles < TM0), tile-major rows 4T+j
                for k, (qlo, qhi) in enumerate(((0, 256), (256, TM0))):
                    if t0 + ntile == qhi:
                        stg = gpool.tile([S, 256], F32, name=f"stg{k}")
                        cp = nc.vector.tensor_copy(
                            stg[:, :qhi - qlo], segP[:, qlo:qhi])
                        for mm in seg_mms:
                            tile.add_dep_helper(cp.ins, mm.ins)
                        dr = nc.gpsimd.dma_start(
                            _dram_view(virt_d, S * qlo,
                                       [[1, S], [S, qhi - qlo]]),
                            stg[:, :qhi - qlo])
                        drains.append(dr)

                if t0 + ntile == TM0:
                    # one run-gather: partition p gets virt[4*t0(p) .. +W)
                    vv = tpool.tile([TIL, W], F32, name="vv")
                    gth = nc.gpsimd.indirect_dma_start(
                        out=vv[:], out_offset=None, in_=virt_d[:],
                        in_offset=bass.IndirectOffsetOnAxis(
                            ap=idx_t[:], axis=0))
                    for dr in drains:
                        tile.add_dep_helper(gth.ins, dr.ins)


            # ---- on-chip combine for tiles >= TM0 (graph rows PB0..127)
            stg4 = gpool.tile([S, NTM], F16, name="stg4")
            cp4 = nc.vector.tensor_copy(stg4[:], segP[:, TM0:NTIL])
            for mm in seg_mms:
                tile.add_dep_helper(cp4.ins, mm.ins)
            ptr = ptail.tile([NTM, S], F16, name="ptr")
            nc.tensor.transpose(ptr[:], stg4[:], pk_t[0:S, 3:7])
            sbT = gpool.tile([NTM, S], F16, name="sbT")
            nc.scalar.activation(sbT[:], ptr[:], AF.Copy)
            for c in range(NCH):
                for j in range(S):
                    blk = NQB * (S * c + j)
                    nc.tensor.matmul(
                        shP[0:NQB, NCH + c:NCH + c + 1],
                        lhsT=mt_t[:, blk:blk + NQB],
                        rhs=sbT[:, j:j + 1],
                        start=(j == 0), stop=(j == S - 1))

            # ---- select partials per graph: s[p,c] = sum_k vv[p,k]*oh[p,c,k]
            # (mult on gpsimd: Pool is idle once its loads finish; fp16
            # intermediates unlock the DVE 2x reduce mode)
            tsel = tpool.tile([TIL, NCH * W], F16, name="tsel")
            nc.gpsimd.tensor_tensor(
                out=tsel[:].rearrange("p (c k) -> p c k", c=NCH),
                in0=_bcast_mid(vv[:], NCH),
                in1=oh_t[:].rearrange("p (c k) -> p c k", c=NCH),
                op=AL.mult)
            s_t = tpool.tile([TIL, NCH], F32, name="s")
            nc.vector.tensor_reduce(
                out=s_t[:],
                in_=tsel[:].rearrange("p (c k) -> p c k", c=NCH),
                axis=mybir.AxisListType.X, op=AL.add)

            # x for all 1024 graphs, then accumulate the on-chip combine
            # into rows >= PB0 in place; single merged logsigmoid + store
            xF = tpool.tile([TIL, NCH], F32, name="xF")
            nc.vector.tensor_add(xF[:], s_t[:], shP[:, 0:NCH])
            if be0 != 0.0:
                xb2 = tpool.tile([TIL, NCH], F32, name="xb2")
                nc.vector.tensor_scalar_add(xb2[:], xF[:], be0)
                xF = xb2
            nc.vector.tensor_add(xF[PB0:TIL, :], xF[PB0:TIL, :], outPB)
            ob = _logsigmoid_chain(nc, tpool, xF[:], TIL, NCH, "F")
            outF = _dram_view(out_d, 0, [[NCH, TIL], [1, NCH]])
            nc.sync.dma_start(outF, ob[:])

            if debug:
                vcp = tpool.tile([TIL, W], F32, name="vcp")
                nc.vector.tensor_copy(vcp[:], vv[:])
                nc.sync.dma_start(vvdbg_d[:, :], vcp[:])
                vload = tpool.tile([TIL, nvc], F32, name="vload")
                gd = nc.gpsimd.dma_start(
                    vload[:], _dram_view(virt_d, 0, [[nvc, TIL], [1, nvc]]))
                for dr in drains:
                    tile.add_dep_helper(gd.ins, dr.ins)
                nc.sync.dma_start(
                    _dram_view(vdbg_d, 0, [[nvc, TIL], [1, nvc]]), vload[:])
                sbc = tpool.tile([NTM, S], F32, name="sbc")
                nc.vector.tensor_copy(sbc[:], sbT[:])
                nc.sync.dma_start(sbtdbg_d[:, :], sbc[:])
    return nc


def _prep_core(hv, seg_ids, last_idx, a, m):
    lo = int(np.searchsorted(seg_ids, m * BL, "left"))
    hi = int(np.searchsorted(seg_ids, (m + 1) * BL, "left"))
    nloc = hi - lo
    assert nloc <= NP - TIL, f"core {m}: {nloc} nodes > capacity"
    seg_loc = seg_ids[lo:hi].astype(np.int64) - m * BL
    sgn = (2 * a[m * BL:(m + 1) * BL] - 1).astype(np.float32)

    hvT = np.zeros((TIL, NP), np.float16)
    hvT[:, :nloc] = hv[lo:hi].astype(np.float16).T

    nrt = (nloc + TIL - 1) // TIL
    b = np.zeros(NTIL, np.int64)
    b[:nrt] = seg_loc[np.arange(nrt) * TIL]
    rel = seg_loc - b[np.arange(nloc) // TIL]
    assert rel.min() >= 0 and rel.max() < S, f"window overflow: {rel.max()}"

    import ml_dtypes
    sel = np.zeros((TIL, NTIL * S), ml_dtypes.float8_e4m3)
    ii = np.arange(nloc)
    sel[ii % TIL, S * (ii // TIL) + rel] = sgn[seg_loc]

    rr = np.arange(BL, dtype=np.int64)
    firsts = np.searchsorted(seg_loc, rr, "left")
    lasts = np.searchsorted(seg_loc, rr + 1, "left")
    nonempty = firsts < lasts
    th = firsts // TIL
    tl = np.maximum(lasts - 1, 0) // TIL
    assert np.all((tl - th)[nonempty] <= 1), "segment spans >2 tiles"
    j1 = rr - b[th]
    assert np.all((j1[nonempty] >= 0) & (j1[nonempty] < S))
    straddle = nonempty & (tl > th)
    assert np.all(b[tl[straddle]] == rr[straddle])
    # graphs on partitions < PB0 (g < 8*PB0) live entirely in tiles < TM0
    assert np.all(tl[nonempty & (rr < NCH * PB0)] < TM0), \
        f"core {m}: early graph owns a late tile"

    # run offsets: partition p covers graphs 8p..8p+7 from tile t0(p)
    t0p = th[NCH * np.arange(TIL)]
    idx = (S * t0p).astype(np.int32).reshape(TIL, 1)
    # one-hot selector oh[p, c, k]: k = 4*(t - t0p) + j for each early
    # partial (home j1 at th; straddle slot 0 at tl)
    oh = np.zeros((TIL, NCH, W), np.float16)
    pp, cc = rr // NCH, rr % NCH
    he = nonempty & (th < TM0)
    k1 = S * (th - t0p[pp]) + j1
    assert np.all((k1[he] >= 0) & (k1[he] < W)), "run width overflow"
    oh[pp[he], cc[he], k1[he]] += 1.0
    se = straddle & (tl < TM0)
    k2 = S * (tl - t0p[pp])
    assert np.all((k2[se] >= 0) & (k2[se] < W)), "run width overflow"
    oh[pp[se], cc[se], k2[se]] += 1.0
    oh = oh.reshape(TIL, NCH * W)

    # M combine for on-chip partials (tiles >= TM0, graphs g >= 8*PB0):
    # block col NQB*(4c + j) + (p - PB0)
    NQB = TIL - PB0
    mt = np.zeros((NTM, NCH * S * NQB), ml_dtypes.float8_e4m3)
    lh = nonempty & (th >= TM0)
    assert np.all(rr[lh] >= NCH * PB0)
    mt[th[lh] - TM0, NQB * (S * cc[lh] + j1[lh]) + pp[lh] - PB0] = 1.0
    ls = straddle & (tl >= TM0)
    assert np.all(rr[ls] >= NCH * PB0)
    mt[tl[ls] - TM0, NQB * (S * cc[ls] + 0) + pp[ls] - PB0] = 1.0

    # srcT column 128c+p holds (sign-folded) src row of graph 8p+c
    src = hv[last_idx[m * BL:(m + 1) * BL]].astype(np.float32) * sgn[:, None]
    srcT = np.zeros((TIL, BL), np.float16)
    gg = np.arange(BL)
    srcT[:, TIL * (gg % NCH) + gg // NCH] = src.T.astype(np.float16)
    return hvT, sel, srcT, mt, oh, idx


def prep_all(hv, Wg, bg, Wp, bp, We, be, seg_ids, last_idx, a):
    hv = np.asarray(hv, dtype=np.float32)
    Wg = np.asarray(Wg, dtype=np.float32)
    bg = np.asarray(bg, dtype=np.float32)
    Wp = np.asarray(Wp, dtype=np.float32)
    bp = np.asarray(bp, dtype=np.float32)
    We = np.asarray(We, dtype=np.float32)
    be = np.asarray(be, dtype=np.float32)
    seg_ids = np.asarray(seg_ids)
    last_idx = np.asarray(last_idx)
    a = np.asarray(a)

    w1 = (Wp @ We[:G]).astype(np.float32)[:, 0]        # [128]
    wes = We[G:, 0].astype(np.float32)                 # [128]
    c1 = float(bp @ We[:G, 0])
    bg0, be0 = float(bg[0]), float(be[0])
    # bp is zeros in this problem's setup_inputs
    assert c1 == 0.0, "c1 != 0 path not implemented"

    pk = np.zeros((TIL, 7), np.float16)
    pk[:, 0] = -Wg[:, 0]
    pk[:, 1] = w1
    pk[:, 2] = wes
    pk[:S, 3:7] = np.eye(S, dtype=np.float16)

    in_maps = []
    for m in range(NCORES):
        hvT, sel, srcT, mt, oh, idx = _prep_core(hv, seg_ids, last_idx, a, m)
        in_maps.append({
            "hvT": hvT, "sel": sel, "srcT": srcT, "mt": mt,
            "oh": oh, "idx": idx, "pk": pk,
        })
    return in_maps, bg0, be0, c1


def _unpermute(out_flat):
    """Device graph order is g = 8p + c stored at flat index 8p+c == g."""
    return out_flat


def kernel(hv, Wg, bg, Wp, bp, We, be, seg_ids, last_idx, a):
    global LAST_RESULTS
    in_maps, bg0, be0, c1 = prep_all(
        hv, Wg, bg, Wp, bp, We, be, seg_ids, last_idx, a)
    nc = _build(bg0, be0, c1)
    split_sync_waits(nc, maxw=1)
    res = run_bass_kernel_spmd(nc, in_maps, core_ids=list(range(NCORES)))
    LAST_RESULTS = res
    out = np.concatenate([np.asarray(res.results[i]["out"]) for i in range(NCORES)], axis=0)
    return out.astype(np.float32)
